# revision 17
# baseline (speedup 1.0000x reference)
"""Trainium2 Bass kernel for NnqlmCnnBasedLstm.

Math (per batch item, per input sequence q/a):
  xe = embed[idx]                      (L, D)       D = 128
  dens_t = outer(xe_t, xe_t)/(|xe_t|^2 + 1e-4)     (D, D), symmetric
  2-layer ConvLSTM over L=40 steps; each gate g:
    pre_g = conv2d([xt; h], W_g, stride=(2,1), pad=(1,1)) + b_g  on (2D, D) -> (D, D)
  c = sig(f)*c + sig(i)*tanh(cc); h = sig(o)*tanh(c)
  out = max_t h2_t  -> flatten -> concat(q,a) -> linear(2) -> log_softmax

Device strategy (8 cores, data parallel over B=32 -> 4 items/core, each with a
q-chain and an a-chain = 8 chains/core):
  * The density inputs are rank-1, so layer 1's ENTIRE x-part contribution
    conv([dens; 0]) is precomputed on the HOST (it only reaches output rows
    j <= 64) and injected into PSUM with one cheap identity matmul per
    (gate, group); the device only convolves the recurrent h rows.
  * State kept TRANSPOSED: tiles are (w partitions, conv-H free).  All
    matmul-path data is fp16 (PE runs 16-bit at 1 col/cycle vs fp32's 1/4)
    with fp32 PSUM accumulation.
  * Combined per-chain segment [P P | h1_{t-1} | h2_{t-2} | P P] (260 cols):
    layer 1 reads the h1 rows of its window, layer 2 reads [h1; h2]
    contiguously; h1 is written exactly once.  Three buffers rotate.
  * conv: out_T[w, j] = sum_{dh,dw} W[dh,dw] * inp_T[w-1+dw, 2j-1+dh].
    For each dh this is a 3-diagonal Toeplitz band matrix (over w) applied on
    the TensorEngine with a stride-2 free-axis access pattern.
  * The 8 chains are split into two GROUPS of 4 processed alternately
    (A-L1, B-L1, A-L2, B-L2): one group's scalar/vector tail (sigmoid/tanh,
    cell update, h write) hides under the other group's matmuls, and each
    (gate, group) PSUM tile is exactly one bank (8 banks total).
  * Embedding gather, final linear + log_softmax on host (tiny).
"""

import os
import sys

import numpy as np

for _p in ("/opt/trn_rl_repo", "/root/.axon_site/_ro/trn_rl_repo"):
    if os.path.isdir(_p) and _p not in sys.path:
        sys.path.insert(0, _p)

B, L, D, V, NL = 32, 40, 128, 32000, 2
NCORES = 8
CH = 8            # chains per core: 4 batch items x {q, a}
SEG = 260         # [P P | h1(128) | h2(128) | P P]
NF = CH * SEG
NJX = 65          # host x-part covers output rows j = 0..64

# layer-1 device dh passes over the h1 rows only: (dh, parity, c-shift, j0, j1)
L1DH = [(0, 1, -64, 65, 128),
        (1, 0, -63, 64, 128),
        (2, 1, -63, 64, 128),
        (3, 0, -62, 63, 127)]
# layer-2 full-range dh passes: (dh, parity, c-shift)
L2DH = [(0, 1, 0), (1, 0, 1), (2, 1, 1), (3, 0, 2)]

_CACHE = {}


def _build_nc(L=L):
    import concourse.bass as bass
    import concourse.bacc as bacc
    import concourse.mybir as mybir
    from concourse import tile

    f32 = mybir.dt.float32
    f16 = mybir.dt.float16
    AF = mybir.ActivationFunctionType
    ALU = mybir.AluOpType

    nc = bacc.Bacc(None, target_bir_lowering=False)

    px_d = nc.dram_tensor("px", (L, D, CH, 4, NJX), f16, kind="ExternalInput")
    st_d = nc.dram_tensor("st", (D, NL * 4 * 4, D), f16, kind="ExternalInput")
    id_d = nc.dram_tensor("ident", (D, D), f16, kind="ExternalInput")
    bias_d = nc.dram_tensor("bias", (D, NL * 4), f32, kind="ExternalInput")
    out_d = nc.dram_tensor("mp_out", (D, CH * D), f16, kind="ExternalOutput")

    GW = 512  # flat columns per group (4 chains x 128)

    with tile.TileContext(nc) as tc:
        with (
            tc.tile_pool(name="const", bufs=1) as constp,
            tc.tile_pool(name="state", bufs=1) as statep,
            tc.tile_pool(name="px", bufs=3) as pxp,
            tc.tile_pool(name="gate", bufs=2) as gatep,
            tc.tile_pool(name="psum", bufs=1, space="PSUM") as psump,
        ):
            # ---- constants ----
            ident = constp.tile([D, D], f16, tag="ident")
            nc.sync.dma_start(ident[:], id_d[:])
            stT = constp.tile([D, NL * 4 * 4 * D], f16, tag="stT")
            nc.sync.dma_start(stT[:], st_d[:].rearrange("p i j -> p (i j)"))
            bias = constp.tile([D, NL * 4], f32, tag="bias")
            nc.sync.dma_start(bias[:], bias_d[:])

            # ---- persistent state ----
            bufs = [statep.tile([D, NF], f16, tag=f"b{i}", name=f"b{i}")
                    for i in range(3)]
            for bt in bufs:
                nc.vector.memset(bt[:], 0.0)
            c_l = [statep.tile([D, CH * D], f16, tag=f"c{l}", name=f"c{l}")
                   for l in range(NL)]
            for l in range(NL):
                nc.vector.memset(c_l[l][:], 0.0)
            mp = statep.tile([D, CH * D], f16, tag="mp")
            nc.vector.memset(mp[:], -60000.0)

            def seg4(t):  # (p, s, c2, two) parity view
                return t[:].rearrange("p (s c two) -> p s c two", s=CH, two=2)

            def segd(t):  # (p, s, c) view for block writes
                return t[:].rearrange("p (s c) -> p s c", s=CH)

            px_tiles = {}

            def dma_px(tt):
                px = pxp.tile([D, CH * 4 * NJX], f16, tag="px", name="px")
                px_tiles[tt] = px
                v = px[:].rearrange("p (s g j) -> p s g j", s=CH, g=4)
                for s in range(CH):
                    nc.sync.dma_start(v[:, s], px_d[tt, :, s])

            def mm_gate_half(l, g, pg, i4, grp, pxv):
                """All matmuls for one (layer, gate, chain-half)."""
                s0 = grp * 4
                pv = pg[:].rearrange("p (s j) -> p s j", s=CH)
                if l == 0:
                    pxview = pxv[:].rearrange(
                        "p (s g j) -> p s g j", s=CH, g=4)
                    nc.tensor.matmul(
                        pv[:, s0:s0 + 4, 0:NJX],
                        ident[:],
                        pxview[:, s0:s0 + 4, g],
                        start=True, stop=False,
                    )
                    for k, (dh, par, csh, j0, j1) in enumerate(L1DH):
                        idx = g * 4 + dh
                        rhs = i4[:, s0:s0 + 4, j0 + csh: j1 + csh, par]
                        nc.tensor.matmul(
                            pv[:, s0:s0 + 4, j0:j1],
                            stT[:, idx * D:(idx + 1) * D],
                            rhs,
                            start=False, stop=(k == 3),
                        )
                else:
                    j0, j1, start0, stop3 = pxv   # (jrange, flags) for l==1
                    for k, (dh, par, csh) in enumerate(L2DH):
                        idx = (4 + g) * 4 + dh
                        rhs = i4[:, s0:s0 + 4, j0 + csh: j1 + csh, par]
                        nc.tensor.matmul(
                            pv[:, s0:s0 + 4, j0:j1],
                            stT[:, idx * D:(idx + 1) * D],
                            rhs,
                            start=(k == 0 and start0),
                            stop=(k == 3 and stop3),
                        )

            def layer1(t, cur, nxt):
                """Layer 1: chain-halves pipelined (h1 feeds layer 2 NOW)."""
                i4 = seg4(cur)
                pxv = px_tiles[t]
                ps = {g: psump.tile([D, CH * D], f32, tag=f"p{g}",
                                    name=f"p{g}") for g in (3, 0, 1, 2)}
                for grp in range(2):
                    csl = slice(grp * GW, (grp + 1) * GW)
                    sl = csl
                    s0 = grp * 4
                    gt = {}
                    for g in (3, 0, 1, 2):
                        mm_gate_half(0, g, ps[g], i4, grp, pxv)
                        bcol = bias[:, g: g + 1]
                        if g == 3:
                            cs = gatep.tile([D, GW], f16, tag=f"cs{grp}",
                                            name=f"cs{grp}")
                            nc.scalar.activation(cs[:], ps[3][:, sl], AF.Tanh,
                                                 bias=bcol)
                            gt[3] = cs
                        elif g == 0:
                            fg = gatep.tile([D, GW], f16, tag=f"fg{grp}",
                                            name=f"fg{grp}")
                            nc.scalar.activation(fg[:], ps[0][:, sl],
                                                 AF.Sigmoid, bias=bcol)
                            t1 = gatep.tile([D, GW], f16, tag=f"t1{grp}",
                                            name=f"t1{grp}")
                            nc.vector.tensor_mul(t1[:], fg[:], c_l[0][:, csl])
                        elif g == 1:
                            ig = gatep.tile([D, GW], f16, tag=f"ig{grp}",
                                            name=f"ig{grp}")
                            nc.scalar.activation(ig[:], ps[1][:, sl],
                                                 AF.Sigmoid, bias=bcol)
                            t2 = gatep.tile([D, GW], f16, tag=f"t2{grp}",
                                            name=f"t2{grp}")
                            nc.vector.tensor_mul(t2[:], ig[:], gt[3][:])
                            nc.vector.tensor_add(c_l[0][:, csl], t1[:], t2[:])
                    og = gatep.tile([D, GW], f16, tag=f"og{grp}",
                                    name=f"og{grp}")
                    nc.scalar.activation(og[:], ps[2][:, sl], AF.Sigmoid,
                                         bias=bias[:, 2:3])
                    th = gatep.tile([D, GW], f16, tag=f"th{grp}",
                                    name=f"th{grp}")
                    nc.scalar.activation(th[:], c_l[0][:, csl], AF.Tanh)
                    hv = segd(nxt)[:, s0:s0 + 4, 2:2 + D]
                    nc.vector.tensor_mul(
                        hv,
                        og[:].rearrange("p (s j) -> p s j", s=4),
                        th[:].rearrange("p (s j) -> p s j", s=4))

            def layer2(t, nxt, nx2):
                """Layer 2: same per-group pipelining; h2/mp are a step from
                being needed, so the tail has slack."""
                i4 = seg4(nxt)
                ps = {g: psump.tile([D, CH * D], f32, tag=f"p{g}",
                                    name=f"p{g}") for g in (3, 0, 1, 2)}
                # early: output rows j>=65 read only h2_{t-1} -- these fill
                # the PE while layer 1's scalar/vector tail completes
                for g in (3, 0, 1, 2):
                    for grp in range(2):
                        mm_gate_half(1, g, ps[g], i4, grp, (65, 128, True, False))
                for grp in range(2):
                    csl = slice(grp * GW, (grp + 1) * GW)
                    s0 = grp * 4
                    gt = {}
                    for g in (3, 0, 1, 2):
                        mm_gate_half(1, g, ps[g], i4, grp, (0, 65, False, True))
                        bcol = bias[:, 4 + g: 4 + g + 1]
                        if g == 3:
                            cs = gatep.tile([D, GW], f16, tag=f"cs{grp}",
                                            name=f"cs{grp}")
                            nc.scalar.activation(cs[:], ps[3][:, csl], AF.Tanh,
                                                 bias=bcol)
                            gt[3] = cs
                        elif g == 0:
                            fg = gatep.tile([D, GW], f16, tag=f"fg{grp}",
                                            name=f"fg{grp}")
                            nc.scalar.activation(fg[:], ps[0][:, csl],
                                                 AF.Sigmoid, bias=bcol)
                            t1 = gatep.tile([D, GW], f16, tag=f"t1{grp}",
                                            name=f"t1{grp}")
                            nc.vector.tensor_mul(t1[:], fg[:], c_l[1][:, csl])
                        elif g == 1:
                            ig = gatep.tile([D, GW], f16, tag=f"ig{grp}",
                                            name=f"ig{grp}")
                            nc.scalar.activation(ig[:], ps[1][:, csl],
                                                 AF.Sigmoid, bias=bcol)
                            t2 = gatep.tile([D, GW], f16, tag=f"t2{grp}",
                                            name=f"t2{grp}")
                            nc.vector.tensor_mul(t2[:], ig[:], gt[3][:])
                            nc.vector.tensor_add(c_l[1][:, csl], t1[:], t2[:])
                    og = gatep.tile([D, GW], f16, tag=f"og{grp}",
                                    name=f"og{grp}")
                    nc.scalar.activation(og[:], ps[2][:, csl], AF.Sigmoid,
                                         bias=bias[:, 6:7])
                    th = gatep.tile([D, GW], f16, tag=f"th{grp}",
                                    name=f"th{grp}")
                    nc.scalar.activation(th[:], c_l[1][:, csl], AF.Tanh)
                    hv = segd(nx2)[:, s0:s0 + 4, 130:130 + D]
                    nc.vector.tensor_mul(
                        hv,
                        og[:].rearrange("p (s j) -> p s j", s=4),
                        th[:].rearrange("p (s j) -> p s j", s=4))
                    mv = mp[:].rearrange("p (s j) -> p s j", s=CH)
                    nc.vector.tensor_tensor(mv[:, s0:s0 + 4, :],
                                            mv[:, s0:s0 + 4, :],
                                            hv, op=ALU.max)

            dma_px(0)
            dma_px(1)

            for t in range(L):
                cur = bufs[t % 3]          # [h1_{t-1}, h2_{t-2}]
                nxt = bufs[(t + 1) % 3]    # gets h1_t
                nx2 = bufs[(t + 2) % 3]    # gets h2_t
                if t + 2 < L:
                    dma_px(t + 2)
                layer1(t, cur, nxt)
                layer2(t, nxt, nx2)
                px_tiles.pop(t, None)

            nc.sync.dma_start(out_d[:], mp[:])

    nc.compile()
    return nc


def _prep_core_inputs(px_all, st, bias_arr, core):
    """px_all: (64, L, 4, D, NJX) fp16 host x-part, chain-major (q0..q31,a0..a31)."""
    qsl = px_all[4 * core:4 * core + 4]
    asl = px_all[32 + 4 * core:32 + 4 * core + 4]
    ch = np.concatenate([qsl, asl], axis=0)        # (8, L, 4, D, NJX)
    # -> (L, D, CH, 4, NJX)
    px = np.ascontiguousarray(ch.transpose(1, 3, 0, 2, 4)).astype(np.float16)
    ident = np.eye(D, dtype=np.float16)
    return {"px": px, "st": st, "bias": bias_arr, "ident": ident}


def kernel(q, a, embed, conv_w, conv_b, lin_w, lin_b):
    from concourse import bass_utils

    q = np.asarray(q); a = np.asarray(a)
    embed = np.asarray(embed, np.float32)
    conv_w = np.asarray(conv_w, np.float32)
    conv_b = np.asarray(conv_b, np.float32)
    lin_w = np.asarray(lin_w, np.float32)
    lin_b = np.asarray(lin_b, np.float32)

    # host: embedding gather + density normalization factors
    idx = np.stack([q, a], axis=1).astype(np.int64)            # (B, 2, L)
    xe = embed[idx].astype(np.float64)                         # (B, 2, L, D)
    dot = np.sum(xe * xe, axis=-1, keepdims=True) + 1e-4
    xe_y = (xe / np.sqrt(dot)).astype(np.float16)

    # host: Toeplitz band stationaries  lhsT[(l,g,dh)] = B^T,
    # B[w, w'] = W[dh, w'-w+1]  (3 diagonals)
    st = np.zeros((NL * 4 * 4, D, D), np.float16)
    for l in range(NL):
        for g in range(4):
            W = conv_w[l, g, 0, 0]                             # (4, 3)
            for dh in range(4):
                Bm = sum(W[dh, dw] * np.eye(D, k=dw - 1) for dw in range(3))
                st[(l * 4 + g) * 4 + dh] = Bm.T.astype(np.float16)
    st = np.ascontiguousarray(st.transpose(1, 0, 2))           # (D, 32, D)
    bias_arr = np.tile(conv_b.reshape(1, -1), (D, 1)).astype(np.float32)

    # host: layer-1 x-part  pre_x[s,t,g][w,j] = sum_dh z[g,dh,w] * yx[dh,j]
    #   z[g,dh,w] = sum_dw W16[g,dh,dw] * y[w-1+dw]   (w-axis pad)
    #   yx[dh,j]  = y[2j-1+dh] where the row index is an x row, else 0
    y = np.concatenate([xe_y[:, 0], xe_y[:, 1]], axis=0).astype(np.float32)
    # y: (64, L, D) chain-major q then a
    W16 = np.zeros((4, 4, 3), np.float32)
    for g in range(4):
        W16[g] = conv_w[0, g, 0, 0].astype(np.float16).astype(np.float32)
    ypad = np.pad(y, ((0, 0), (0, 0), (1, 1)))                 # (64, L, D+2)
    z = np.einsum('gdv,stwv->stgdw', W16,
                  np.stack([ypad[:, :, dw:dw + D] for dw in range(3)], -1))
    jj = np.arange(NJX)
    yx = np.zeros((64, L, 4, NJX), np.float32)
    for dh in range(4):
        r = 2 * jj - 1 + dh
        ok = (r >= 0) & (r < D)
        yx[:, :, dh, ok] = y[:, :, r[ok]]
    px_all = np.einsum('stgdw,stdj->stgwj', z, yx).astype(np.float16)
    # px_all: (64, L, 4, D, NJX)
    px_all = np.ascontiguousarray(px_all.transpose(0, 1, 2, 3, 4))

    if "nc" not in _CACHE:
        _CACHE["nc"] = _build_nc()
    nc = _CACHE["nc"]

    in_maps = [_prep_core_inputs(px_all, st, bias_arr, i) for i in range(NCORES)]
    _CACHE["in_maps"] = in_maps
    res = bass_utils.run_bass_kernel_spmd(nc, in_maps, core_ids=list(range(NCORES)))

    # host: unshard + final linear + log_softmax
    q_p = np.zeros((B, D * D), np.float32)
    a_p = np.zeros((B, D * D), np.float32)
    for i in range(NCORES):
        out = res.results[i]["mp_out"]                         # (D w, CH*D)
        for s in range(CH):
            mp_T = out[:, s * D:(s + 1) * D].astype(np.float32)  # (w, j)
            flat = np.ascontiguousarray(mp_T.T).reshape(-1)    # j-major
            if s < 4:
                q_p[4 * i + s] = flat
            else:
                a_p[4 * i + s - 4] = flat
    qa = np.concatenate([q_p, a_p], axis=1)
    score = qa @ lin_w.T + lin_b
    m = score.max(axis=1, keepdims=True)
    ls = score - m
    lse = np.log(np.exp(ls).sum(axis=1, keepdims=True))
    return (ls - lse).astype(np.float32)


# revision 18
# speedup vs baseline: 1.0348x; 1.0348x over previous
"""Trainium2 Bass kernel for NnqlmCnnBasedLstm.

Math (per batch item, per input sequence q/a):
  xe = embed[idx]                      (L, D)       D = 128
  dens_t = outer(xe_t, xe_t)/(|xe_t|^2 + 1e-4)     (D, D), symmetric
  2-layer ConvLSTM over L=40 steps; each gate g:
    pre_g = conv2d([xt; h], W_g, stride=(2,1), pad=(1,1)) + b_g  on (2D, D) -> (D, D)
  c = sig(f)*c + sig(i)*tanh(cc); h = sig(o)*tanh(c)
  out = max_t h2_t  -> flatten -> concat(q,a) -> linear(2) -> log_softmax

Device strategy (8 cores, data parallel over B=32 -> 4 items/core, each with a
q-chain and an a-chain = 8 chains/core):
  * The density inputs are rank-1, so layer 1's ENTIRE x-part contribution
    conv([dens; 0]) is precomputed on the HOST (it only reaches output rows
    j <= 64) and injected into PSUM with one cheap identity matmul per
    (gate, group); the device only convolves the recurrent h rows.
  * State kept TRANSPOSED: tiles are (w partitions, conv-H free).  All
    matmul-path data is fp16 (PE runs 16-bit at 1 col/cycle vs fp32's 1/4)
    with fp32 PSUM accumulation.
  * Combined per-chain segment [P P | h1_{t-1} | h2_{t-2} | P P] (260 cols):
    layer 1 reads the h1 rows of its window, layer 2 reads [h1; h2]
    contiguously; h1 is written exactly once.  Three buffers rotate.
  * conv: out_T[w, j] = sum_{dh,dw} W[dh,dw] * inp_T[w-1+dw, 2j-1+dh].
    For each dh this is a 3-diagonal Toeplitz band matrix (over w) applied on
    the TensorEngine with a stride-2 free-axis access pattern.
  * The 8 chains are split into two GROUPS of 4 processed alternately
    (A-L1, B-L1, A-L2, B-L2): one group's scalar/vector tail (sigmoid/tanh,
    cell update, h write) hides under the other group's matmuls, and each
    (gate, group) PSUM tile is exactly one bank (8 banks total).
  * Embedding gather, final linear + log_softmax on host (tiny).
"""

import os
import sys

import numpy as np

for _p in ("/opt/trn_rl_repo", "/root/.axon_site/_ro/trn_rl_repo"):
    if os.path.isdir(_p) and _p not in sys.path:
        sys.path.insert(0, _p)

B, L, D, V, NL = 32, 40, 128, 32000, 2
NCORES = 8
CH = 8            # chains per core: 4 batch items x {q, a}
SEG = 260         # [P P | h1(128) | h2(128) | P P]
NF = CH * SEG
NJX = 65          # host x-part covers output rows j = 0..64

# layer-1 device dh passes over the h1 rows only: (dh, parity, c-shift, j0, j1)
L1DH = [(0, 1, -64, 65, 128),
        (1, 0, -63, 64, 128),
        (2, 1, -63, 64, 128),
        (3, 0, -62, 63, 127)]
# layer-2 full-range dh passes: (dh, parity, c-shift)
L2DH = [(0, 1, 0), (1, 0, 1), (2, 1, 1), (3, 0, 2)]

_CACHE = {}


def _build_nc(L=L):
    import concourse.bass as bass
    import concourse.bacc as bacc
    import concourse.mybir as mybir
    from concourse import tile

    f32 = mybir.dt.float32
    f16 = mybir.dt.float16
    AF = mybir.ActivationFunctionType
    ALU = mybir.AluOpType

    nc = bacc.Bacc(None, target_bir_lowering=False)

    px_d = nc.dram_tensor("px", (L, D, CH, 4, NJX), f16, kind="ExternalInput")
    st_d = nc.dram_tensor("st", (D, NL * 4 * 4, D), f16, kind="ExternalInput")
    id_d = nc.dram_tensor("ident", (D, D), f16, kind="ExternalInput")
    bias_d = nc.dram_tensor("bias", (D, NL * 4), f32, kind="ExternalInput")
    out_d = nc.dram_tensor("mp_out", (D, CH * D), f16, kind="ExternalOutput")

    GW = 512  # flat columns per group (4 chains x 128)

    with tile.TileContext(nc) as tc:
        with (
            tc.tile_pool(name="const", bufs=1) as constp,
            tc.tile_pool(name="state", bufs=1) as statep,
            tc.tile_pool(name="px", bufs=3) as pxp,
            tc.tile_pool(name="gate", bufs=2) as gatep,
            tc.tile_pool(name="psum", bufs=1, space="PSUM") as psump,
        ):
            # ---- constants ----
            ident = constp.tile([D, D], f16, tag="ident")
            nc.sync.dma_start(ident[:], id_d[:])
            stT = constp.tile([D, NL * 4 * 4 * D], f16, tag="stT")
            nc.sync.dma_start(stT[:], st_d[:].rearrange("p i j -> p (i j)"))
            bias = constp.tile([D, NL * 4], f32, tag="bias")
            nc.sync.dma_start(bias[:], bias_d[:])

            # ---- persistent state ----
            bufs = [statep.tile([D, NF], f16, tag=f"b{i}", name=f"b{i}")
                    for i in range(3)]
            for bt in bufs:
                nc.vector.memset(bt[:], 0.0)
            c_l = [statep.tile([D, CH * D], f16, tag=f"c{l}", name=f"c{l}")
                   for l in range(NL)]
            for l in range(NL):
                nc.vector.memset(c_l[l][:], 0.0)
            mp = statep.tile([D, CH * D], f16, tag="mp")
            nc.vector.memset(mp[:], -60000.0)

            def seg4(t):  # (p, s, c2, two) parity view
                return t[:].rearrange("p (s c two) -> p s c two", s=CH, two=2)

            def segd(t):  # (p, s, c) view for block writes
                return t[:].rearrange("p (s c) -> p s c", s=CH)

            px_tiles = {}

            def dma_px(tt):
                px = pxp.tile([D, CH * 4 * NJX], f16, tag="px", name="px")
                px_tiles[tt] = px
                v = px[:].rearrange("p (s g j) -> p s g j", s=CH, g=4)
                for s in range(CH):
                    nc.sync.dma_start(v[:, s], px_d[tt, :, s])

            def mm_gate_half(l, g, pg, i4, grp, pxv):
                """All matmuls for one (layer, gate, chain-half)."""
                s0 = grp * 4
                pv = pg[:].rearrange("p (s j) -> p s j", s=CH)
                if l == 0:
                    pxview = pxv[:].rearrange(
                        "p (s g j) -> p s g j", s=CH, g=4)
                    nc.tensor.matmul(
                        pv[:, s0:s0 + 4, 0:NJX],
                        ident[:],
                        pxview[:, s0:s0 + 4, g],
                        start=True, stop=False,
                    )
                    for k, (dh, par, csh, j0, j1) in enumerate(L1DH):
                        idx = g * 4 + dh
                        rhs = i4[:, s0:s0 + 4, j0 + csh: j1 + csh, par]
                        nc.tensor.matmul(
                            pv[:, s0:s0 + 4, j0:j1],
                            stT[:, idx * D:(idx + 1) * D],
                            rhs,
                            start=False, stop=(k == 3),
                        )
                else:
                    j0, j1, start0, stop3 = pxv   # (jrange, flags) for l==1
                    for k, (dh, par, csh) in enumerate(L2DH):
                        idx = (4 + g) * 4 + dh
                        rhs = i4[:, s0:s0 + 4, j0 + csh: j1 + csh, par]
                        nc.tensor.matmul(
                            pv[:, s0:s0 + 4, j0:j1],
                            stT[:, idx * D:(idx + 1) * D],
                            rhs,
                            start=(k == 0 and start0),
                            stop=(k == 3 and stop3),
                        )

            def layer1(t, cur, nxt):
                """Layer 1: chain-halves pipelined (h1 feeds layer 2 NOW)."""
                i4 = seg4(cur)
                pxv = px_tiles[t]
                ps = {g: psump.tile([D, CH * D], f32, tag=f"p{g}",
                                    name=f"p{g}") for g in (3, 0, 1, 2)}
                for grp in range(2):
                    csl = slice(grp * GW, (grp + 1) * GW)
                    sl = csl
                    s0 = grp * 4
                    gt = {}
                    for g in (3, 0, 1, 2):
                        mm_gate_half(0, g, ps[g], i4, grp, pxv)
                        bcol = bias[:, g: g + 1]
                        if g == 3:
                            cs = gatep.tile([D, GW], f16, tag=f"cs{grp}",
                                            name=f"cs{grp}")
                            nc.scalar.activation(cs[:], ps[3][:, sl], AF.Tanh,
                                                 bias=bcol)
                            gt[3] = cs
                        elif g == 0:
                            fg = gatep.tile([D, GW], f16, tag=f"fg{grp}",
                                            name=f"fg{grp}")
                            nc.scalar.activation(fg[:], ps[0][:, sl],
                                                 AF.Sigmoid, bias=bcol)
                            t1 = gatep.tile([D, GW], f16, tag=f"t1{grp}",
                                            name=f"t1{grp}")
                            nc.vector.tensor_mul(t1[:], fg[:], c_l[0][:, csl])
                        elif g == 1:
                            ig = gatep.tile([D, GW], f16, tag=f"ig{grp}",
                                            name=f"ig{grp}")
                            nc.scalar.activation(ig[:], ps[1][:, sl],
                                                 AF.Sigmoid, bias=bcol)
                            t2 = gatep.tile([D, GW], f16, tag=f"t2{grp}",
                                            name=f"t2{grp}")
                            nc.vector.tensor_mul(t2[:], ig[:], gt[3][:])
                            nc.vector.tensor_add(c_l[0][:, csl], t1[:], t2[:])
                    og = gatep.tile([D, GW], f16, tag=f"og{grp}",
                                    name=f"og{grp}")
                    nc.scalar.activation(og[:], ps[2][:, sl], AF.Sigmoid,
                                         bias=bias[:, 2:3])
                    th = gatep.tile([D, GW], f16, tag=f"th{grp}",
                                    name=f"th{grp}")
                    nc.scalar.activation(th[:], c_l[0][:, csl], AF.Tanh)
                    hv = segd(nxt)[:, s0:s0 + 4, 2:2 + D]
                    nc.vector.tensor_mul(
                        hv,
                        og[:].rearrange("p (s j) -> p s j", s=4),
                        th[:].rearrange("p (s j) -> p s j", s=4))

            def layer2(t, nxt, nx2):
                """Layer 2: same per-group pipelining; h2/mp are a step from
                being needed, so the tail has slack."""
                i4 = seg4(nxt)
                ps = {g: psump.tile([D, CH * D], f32, tag=f"p{g}",
                                    name=f"p{g}") for g in (3, 0, 1, 2)}
                for grp in range(2):
                    csl = slice(grp * GW, (grp + 1) * GW)
                    s0 = grp * 4
                    gt = {}
                    for g in (3, 0, 1, 2):
                        mm_gate_half(1, g, ps[g], i4, grp, (0, 128, True, True))
                        bcol = bias[:, 4 + g: 4 + g + 1]
                        if g == 3:
                            cs = gatep.tile([D, GW], f16, tag=f"cs{grp}",
                                            name=f"cs{grp}")
                            nc.scalar.activation(cs[:], ps[3][:, csl], AF.Tanh,
                                                 bias=bcol)
                            gt[3] = cs
                        elif g == 0:
                            fg = gatep.tile([D, GW], f16, tag=f"fg{grp}",
                                            name=f"fg{grp}")
                            nc.scalar.activation(fg[:], ps[0][:, csl],
                                                 AF.Sigmoid, bias=bcol)
                            t1 = gatep.tile([D, GW], f16, tag=f"t1{grp}",
                                            name=f"t1{grp}")
                            nc.vector.tensor_mul(t1[:], fg[:], c_l[1][:, csl])
                        elif g == 1:
                            ig = gatep.tile([D, GW], f16, tag=f"ig{grp}",
                                            name=f"ig{grp}")
                            nc.scalar.activation(ig[:], ps[1][:, csl],
                                                 AF.Sigmoid, bias=bcol)
                            t2 = gatep.tile([D, GW], f16, tag=f"t2{grp}",
                                            name=f"t2{grp}")
                            nc.vector.tensor_mul(t2[:], ig[:], gt[3][:])
                            nc.vector.tensor_add(c_l[1][:, csl], t1[:], t2[:])
                    og = gatep.tile([D, GW], f16, tag=f"og{grp}",
                                    name=f"og{grp}")
                    nc.scalar.activation(og[:], ps[2][:, csl], AF.Sigmoid,
                                         bias=bias[:, 6:7])
                    th = gatep.tile([D, GW], f16, tag=f"th{grp}",
                                    name=f"th{grp}")
                    nc.scalar.activation(th[:], c_l[1][:, csl], AF.Tanh)
                    hv = segd(nx2)[:, s0:s0 + 4, 130:130 + D]
                    nc.vector.tensor_mul(
                        hv,
                        og[:].rearrange("p (s j) -> p s j", s=4),
                        th[:].rearrange("p (s j) -> p s j", s=4))
                    mv = mp[:].rearrange("p (s j) -> p s j", s=CH)
                    nc.vector.tensor_tensor(mv[:, s0:s0 + 4, :],
                                            mv[:, s0:s0 + 4, :],
                                            hv, op=ALU.max)

            dma_px(0)
            dma_px(1)

            for t in range(L):
                cur = bufs[t % 3]          # [h1_{t-1}, h2_{t-2}]
                nxt = bufs[(t + 1) % 3]    # gets h1_t
                nx2 = bufs[(t + 2) % 3]    # gets h2_t
                if t + 2 < L:
                    dma_px(t + 2)
                layer1(t, cur, nxt)
                layer2(t, nxt, nx2)
                px_tiles.pop(t, None)

            nc.sync.dma_start(out_d[:], mp[:])

    nc.compile()
    return nc


def _prep_core_inputs(px_all, st, bias_arr, core):
    """px_all: (64, L, 4, D, NJX) fp16 host x-part, chain-major (q0..q31,a0..a31)."""
    qsl = px_all[4 * core:4 * core + 4]
    asl = px_all[32 + 4 * core:32 + 4 * core + 4]
    ch = np.concatenate([qsl, asl], axis=0)        # (8, L, 4, D, NJX)
    # -> (L, D, CH, 4, NJX)
    px = np.ascontiguousarray(ch.transpose(1, 3, 0, 2, 4)).astype(np.float16)
    ident = np.eye(D, dtype=np.float16)
    return {"px": px, "st": st, "bias": bias_arr, "ident": ident}


def kernel(q, a, embed, conv_w, conv_b, lin_w, lin_b):
    from concourse import bass_utils

    q = np.asarray(q); a = np.asarray(a)
    embed = np.asarray(embed, np.float32)
    conv_w = np.asarray(conv_w, np.float32)
    conv_b = np.asarray(conv_b, np.float32)
    lin_w = np.asarray(lin_w, np.float32)
    lin_b = np.asarray(lin_b, np.float32)

    # host: embedding gather + density normalization factors
    idx = np.stack([q, a], axis=1).astype(np.int64)            # (B, 2, L)
    xe = embed[idx].astype(np.float64)                         # (B, 2, L, D)
    dot = np.sum(xe * xe, axis=-1, keepdims=True) + 1e-4
    xe_y = (xe / np.sqrt(dot)).astype(np.float16)

    # host: Toeplitz band stationaries  lhsT[(l,g,dh)] = B^T,
    # B[w, w'] = W[dh, w'-w+1]  (3 diagonals)
    st = np.zeros((NL * 4 * 4, D, D), np.float16)
    for l in range(NL):
        for g in range(4):
            W = conv_w[l, g, 0, 0]                             # (4, 3)
            for dh in range(4):
                Bm = sum(W[dh, dw] * np.eye(D, k=dw - 1) for dw in range(3))
                st[(l * 4 + g) * 4 + dh] = Bm.T.astype(np.float16)
    st = np.ascontiguousarray(st.transpose(1, 0, 2))           # (D, 32, D)
    bias_arr = np.tile(conv_b.reshape(1, -1), (D, 1)).astype(np.float32)

    # host: layer-1 x-part  pre_x[s,t,g][w,j] = sum_dh z[g,dh,w] * yx[dh,j]
    #   z[g,dh,w] = sum_dw W16[g,dh,dw] * y[w-1+dw]   (w-axis pad)
    #   yx[dh,j]  = y[2j-1+dh] where the row index is an x row, else 0
    y = np.concatenate([xe_y[:, 0], xe_y[:, 1]], axis=0).astype(np.float32)
    # y: (64, L, D) chain-major q then a
    W16 = np.zeros((4, 4, 3), np.float32)
    for g in range(4):
        W16[g] = conv_w[0, g, 0, 0].astype(np.float16).astype(np.float32)
    ypad = np.pad(y, ((0, 0), (0, 0), (1, 1)))                 # (64, L, D+2)
    z = np.einsum('gdv,stwv->stgdw', W16,
                  np.stack([ypad[:, :, dw:dw + D] for dw in range(3)], -1))
    jj = np.arange(NJX)
    yx = np.zeros((64, L, 4, NJX), np.float32)
    for dh in range(4):
        r = 2 * jj - 1 + dh
        ok = (r >= 0) & (r < D)
        yx[:, :, dh, ok] = y[:, :, r[ok]]
    px_all = np.einsum('stgdw,stdj->stgwj', z, yx).astype(np.float16)
    # px_all: (64, L, 4, D, NJX)
    px_all = np.ascontiguousarray(px_all.transpose(0, 1, 2, 3, 4))

    if "nc" not in _CACHE:
        _CACHE["nc"] = _build_nc()
    nc = _CACHE["nc"]

    in_maps = [_prep_core_inputs(px_all, st, bias_arr, i) for i in range(NCORES)]
    _CACHE["in_maps"] = in_maps
    res = bass_utils.run_bass_kernel_spmd(nc, in_maps, core_ids=list(range(NCORES)))

    # host: unshard + final linear + log_softmax
    q_p = np.zeros((B, D * D), np.float32)
    a_p = np.zeros((B, D * D), np.float32)
    for i in range(NCORES):
        out = res.results[i]["mp_out"]                         # (D w, CH*D)
        for s in range(CH):
            mp_T = out[:, s * D:(s + 1) * D].astype(np.float32)  # (w, j)
            flat = np.ascontiguousarray(mp_T.T).reshape(-1)    # j-major
            if s < 4:
                q_p[4 * i + s] = flat
            else:
                a_p[4 * i + s - 4] = flat
    qa = np.concatenate([q_p, a_p], axis=1)
    score = qa @ lin_w.T + lin_b
    m = score.max(axis=1, keepdims=True)
    ls = score - m
    lse = np.log(np.exp(ls).sum(axis=1, keepdims=True))
    return (ls - lse).astype(np.float32)
